# revision 1
# baseline (speedup 1.0000x reference)
"""MoE ConditionalFeedForward (int8 SwiGLU experts) on 8 trn2 NeuronCores.

Strategy: expert-parallel. Host routes token(+slot) pairs to their expert,
pads each expert's token batch to a common capacity C, pre-tiles the weights
into the exact contiguous chunks the kernel DMAs (so every DMA is one large
fully-contiguous read ~1-2MB, measured ~360GB/s/core), and ships one expert
per core. w1 is pre-converted to fp16 on the host (phase A engines are busy);
w3/w2 ship as int8 and are cast to fp16 on-chip by ACT/DVE/POOL, overlapped
with the TensorEngine:

    h1^T[i,c] = sum_d w1[e][i,d] * x[c,d]        (PE, fp16, fp32 accum)
    a = Silu(h1^T * s1)                           (ACT, per-partition scale)
    b = h3^T * s3                                 (ACT copy w/ scale)
    h = a * b                                     (DVE, fp16)
    y^T[m,c] = sum_i w2[e][m,i] * h[i,c]          (PE)
    out = y^T * s2                                (DVE)

Host then scatters each expert's [C, D] result back to out[t, a, :].
"""

import os

import numpy as np

os.environ.setdefault("JAX_COMPILATION_CACHE_DIR", "/tmp/jax_cache")

# Problem constants (hardcoded per the task contract).
E = 8
D = 2048
I = 7168
P = 128
GW = 512        # phase A i-group width
PBM = 8         # phase B m-tiles per group (PSUM banks)
PBI = 4         # phase B i-tiles per DMA chunk

_CACHE = {}
_LAST_RESULTS = None  # for test harness introspection

# measured per-engine int8->fp16 cast rates (elems/ns, 128 partitions)
_RATE = {"act": 0.090, "dve": 0.061, "pool": 0.028}


def _build_nc(C, d=D, i_dim=I, gw=GW, pbm=PBM, pbi=PBI, use_silu=True):
    import concourse.bacc as bacc
    import concourse.tile as tile
    from concourse import mybir

    f16 = mybir.dt.float16
    f32 = mybir.dt.float32
    i8 = mybir.dt.int8

    KD = d // P          # contraction tiles for GEMM1/3
    KI = i_dim // P      # contraction tiles for GEMM2 / feature tiles
    assert i_dim % gw == 0
    NG = i_dim // gw     # phase A weight groups
    IL = gw // P         # i-tiles per group
    MT = d // P          # output m-tiles
    assert MT % pbm == 0
    MH = MT // pbm       # phase B m-groups
    PBW = pbm * P        # phase B weight chunk width (d)
    assert KI % pbi == 0
    NB = KI // pbi       # phase B chunks per m-group

    nc = bacc.Bacc("TRN2", target_bir_lowering=False, debug=False, num_devices=E)

    xt = nc.dram_tensor("xt", [d, C], f16, kind="ExternalInput").ap()
    # pre-tiled weights: partition-major so every DMA descriptor is a long
    # contiguous per-partition run (16KB for w1 -> ~360GB/s measured)
    w1t = nc.dram_tensor("w1t", [NG, P, KD * gw], f16, kind="ExternalInput").ap()
    w3t = nc.dram_tensor("w3t", [NG, P, KD * gw], i8, kind="ExternalInput").ap()
    w2t = nc.dram_tensor("w2t", [MH, P, KI * PBW], i8, kind="ExternalInput").ap()
    s1 = nc.dram_tensor("s1", [P, KI], f32, kind="ExternalInput").ap()
    s3 = nc.dram_tensor("s3", [P, KI], f32, kind="ExternalInput").ap()
    s2 = nc.dram_tensor("s2", [P, MT], f32, kind="ExternalInput").ap()
    yt = nc.dram_tensor("yt", [d, C], f32, kind="ExternalOutput").ap()

    with tile.TileContext(nc) as tc:
        # weighted engine picker for cast/elementwise work: assign each op to
        # the engine that would finish it soonest at measured rates.
        acc = {"act": 0.0, "dve": 0.0, "pool": 0.0}

        def pick(elems, engines=("act", "dve", "pool")):
            best = min(engines, key=lambda e: acc[e] + elems / _RATE[e])
            acc[best] += elems / _RATE[best]
            return best

        def cast(out, in_, engines=("act", "dve", "pool")):
            eng = pick(out.shape[0] * out.free_size() / P * 128, engines)
            if eng == "act":
                nc.scalar.copy(out, in_)
            elif eng == "dve":
                nc.vector.tensor_copy(out, in_)
            else:
                nc.gpsimd.tensor_copy(out, in_)

        import contextlib

        with contextlib.ExitStack() as ctx:
            constp = ctx.enter_context(tc.tile_pool(name="const", bufs=1))
            w1p = ctx.enter_context(tc.tile_pool(name="w1p", bufs=2))
            stagep = ctx.enter_context(tc.tile_pool(name="stage", bufs=2))
            wfp = ctx.enter_context(tc.tile_pool(name="wf", bufs=2))
            hp = ctx.enter_context(tc.tile_pool(name="h", bufs=1))
            ep = ctx.enter_context(tc.tile_pool(name="eltw", bufs=3))
            outp = ctx.enter_context(tc.tile_pool(name="outp", bufs=4))

            # Constants: x^T (fp16) and the scale vectors.
            xts = constp.tile([P, KD, C], f16)
            nc.sync.dma_start(xts, xt.rearrange("(ko p) c -> p ko c", p=P))
            s1s = constp.tile([P, KI], f32)
            nc.sync.dma_start(s1s, s1)
            s3s = constp.tile([P, KI], f32)
            nc.sync.dma_start(s3s, s3)
            s2s = constp.tile([P, MT], f32)
            nc.sync.dma_start(s2s, s2)

            h_tiles = []

            # ---------------- Phase A: h = silu(x@w1^T * s1) * (x@w3^T * s3)
            with tc.tile_pool(name="psA", bufs=2, space="PSUM") as psA:
                for g in range(NG):
                    # one contiguous DMA per matrix per group
                    w1sb = w1p.tile([P, KD, gw], f16, tag="w1sb")
                    nc.sync.dma_start(
                        w1sb, w1t[g].rearrange("p (k f) -> p k f", f=gw)
                    )
                    w3s8 = stagep.tile([P, KD, gw], i8, tag="w3s8")
                    nc.sync.dma_start(
                        w3s8, w3t[g].rearrange("p (k f) -> p k f", f=gw)
                    )
                    w3f = []
                    for k in range(KD):
                        w3fk = wfp.tile([P, gw], f16, tag=f"w3f_{k}")
                        cast(w3fk, w3s8[:, k, :])
                        w3f.append(w3fk)
                    for il in range(IL):
                        i = g * IL + il
                        p1 = psA.tile([P, C], f32, tag="p1")
                        p3 = psA.tile([P, C], f32, tag="p3")
                        for k in range(KD):
                            nc.tensor.matmul(
                                p1, w1sb[:, k, il * P:(il + 1) * P],
                                xts[:, k, :], start=(k == 0), stop=(k == KD - 1),
                            )
                        for k in range(KD):
                            nc.tensor.matmul(
                                p3, w3f[k][:, il * P:(il + 1) * P],
                                xts[:, k, :], start=(k == 0), stop=(k == KD - 1),
                            )
                        htile = hp.tile([P, C], f16, tag=f"h{i}")
                        b = ep.tile([P, C], f16, tag="b")
                        nc.scalar.mul(b, p3, s3s[:, i:i + 1])
                        acc["act"] += C * 0.9
                        if use_silu:
                            a = ep.tile([P, C], f16, tag="a")
                            nc.scalar.activation(
                                a, p1, mybir.ActivationFunctionType.Silu,
                                scale=s1s[:, i:i + 1],
                            )
                            acc["act"] += C * 0.9
                            nc.vector.tensor_mul(htile, a, b)
                            acc["dve"] += C * 1.5
                        else:
                            # Simulator-friendly decomposition:
                            # silu(v) = v * sigmoid(v), v = p1*s1
                            sg = ep.tile([P, C], f32, tag="sg")
                            nc.scalar.activation(
                                sg, p1, mybir.ActivationFunctionType.Sigmoid,
                                scale=s1s[:, i:i + 1],
                            )
                            v = ep.tile([P, C], f32, tag="v")
                            nc.vector.tensor_scalar_mul(v, p1, s1s[:, i:i + 1])
                            ab = ep.tile([P, C], f16, tag="ab")
                            nc.vector.tensor_mul(ab, sg, v)
                            nc.vector.tensor_mul(htile, ab, b)
                        h_tiles.append(htile)

            # ---------------- Phase B: y^T = w2 @ h, scaled by s2
            with tc.tile_pool(name="psB", bufs=1, space="PSUM") as psB:
                for mh in range(MH):
                    pbs = [
                        psB.tile([P, C], f32, tag=f"pb{ml}", name=f"pb{mh}_{ml}")
                        for ml in range(pbm)
                    ]
                    for nb in range(NB):
                        w2s8 = stagep.tile([P, pbi, PBW], i8, tag="w2s8", bufs=3)
                        nc.sync.dma_start(
                            w2s8,
                            w2t[mh][:, nb * pbi * PBW:(nb + 1) * pbi * PBW]
                            .rearrange("p (i f) -> p i f", f=PBW),
                        )
                        w2f = wfp.tile([P, pbi, PBW], f16, tag="w2f", bufs=3)
                        for i_l in range(pbi):
                            cast(w2f[:, i_l, :], w2s8[:, i_l, :])
                        for i_l in range(pbi):
                            i = nb * pbi + i_l
                            for ml in range(pbm):
                                nc.tensor.matmul(
                                    pbs[ml], w2f[:, i_l, ml * P:(ml + 1) * P],
                                    h_tiles[i],
                                    start=(i == 0), stop=(i == KI - 1),
                                )
                    for ml in range(pbm):
                        m = mh * pbm + ml
                        o = outp.tile([P, C], f32, tag="o")
                        nc.vector.tensor_scalar_mul(o, pbs[ml], s2s[:, m:m + 1])
                        nc.sync.dma_start(yt[m * P:(m + 1) * P, :], o)

    nc.compile()
    return nc


def _tile_w13(w, gw=GW):
    """[I', D'] -> [NG, P, KD*gw] partition-major (chunk[g][p][k*gw+f] =
    w[g*gw+f, k*P+p])."""
    i_dim, d = w.shape
    return np.ascontiguousarray(
        w.reshape(i_dim // gw, gw, d // P, P).transpose(0, 3, 2, 1)
    ).reshape(i_dim // gw, P, d // P * gw)


def _tile_w2(w, pbm=PBM):
    """[D', I'] -> [MH, P, KI*PBW] partition-major (chunk[mh][p][i*pbw+f] =
    w[mh*pbw+f, i*P+p])."""
    d, i_dim = w.shape
    pbw = pbm * P
    return np.ascontiguousarray(
        w.reshape(d // pbw, pbw, i_dim // P, P).transpose(0, 3, 2, 1)
    ).reshape(d // pbw, P, i_dim // P * pbw)


def _route(expert_indices):
    """Group (token, slot) pairs by expert. Returns per-expert index arrays."""
    idx = np.asarray(expert_indices).astype(np.int64)
    toks = []
    slots = []
    for e in range(E):
        t, a = np.nonzero(idx == e)
        toks.append(t)
        slots.append(a)
    return toks, slots


def _prepare(inputs):
    """Route tokens, build per-core input maps, return (nc, in_maps, meta)."""
    x = np.asarray(inputs["x"], dtype=np.float32)          # [T, D]
    expert_indices = np.asarray(inputs["expert_indices"])  # [T, A]
    w1 = np.asarray(inputs["w1"])                          # [E, I, D] int8
    w2 = np.asarray(inputs["w2"])                          # [E, D, I] int8
    w3 = np.asarray(inputs["w3"])                          # [E, I, D] int8
    scales1 = np.asarray(inputs["scales1"], dtype=np.float32)  # [E, I]
    scales2 = np.asarray(inputs["scales2"], dtype=np.float32)  # [E, D]
    scales3 = np.asarray(inputs["scales3"], dtype=np.float32)  # [E, I]

    T, A = expert_indices.shape
    toks, slots = _route(expert_indices)
    counts = [len(t) for t in toks]
    C = max(max(counts), 8)
    C = (C + 1) // 2 * 2

    if C not in _CACHE:
        _CACHE[C] = _build_nc(C)
    nc = _CACHE[C]

    KI = I // P
    MT = D // P
    in_maps = []
    for e in range(E):
        n_e = counts[e]
        xtc = np.zeros((D, C), dtype=np.float16)
        if n_e:
            xtc[:, :n_e] = x[toks[e]].T.astype(np.float16)
        in_maps.append(
            dict(
                xt=xtc,
                w1t=_tile_w13(w1[e].astype(np.float16)),
                w3t=_tile_w13(w3[e]),
                w2t=_tile_w2(w2[e]),
                s1=np.ascontiguousarray(scales1[e].reshape(KI, P).T),
                s3=np.ascontiguousarray(scales3[e].reshape(KI, P).T),
                s2=np.ascontiguousarray(scales2[e].reshape(MT, P).T),
            )
        )
    return nc, in_maps, (T, A, toks, slots, counts)


def kernel(**inputs):
    global _LAST_RESULTS
    from concourse.bass_utils import run_bass_kernel_spmd

    nc, in_maps, (T, A, toks, slots, counts) = _prepare(inputs)
    res = run_bass_kernel_spmd(nc, in_maps, core_ids=list(range(E)))
    _LAST_RESULTS = res

    out = np.zeros((T, A, D), dtype=np.float32)
    for e in range(E):
        n_e = counts[e]
        if n_e:
            ye = res.results[e]["yt"][:, :n_e].T  # [n_e, D]
            out[toks[e], slots[e], :] = ye
    return out

